# revision 1
# baseline (speedup 1.0000x reference)
"""Trainium2 Bass kernel for nn_CustomConvLayer (dynamic per-sample conv).

Sharding: pure data parallel over batch B=32 across 8 NeuronCores (4
samples per core). Small synthesis networks are replicated per core.

Per sample, on device:
  wm-embedding MLP -> per-channel modulation wm_coff  (tiny matmuls)
  t0 = 2x2 sum-pool(x)                                (DVE window reduce)
  tower: 3 convs (lrelu) -> t3                        (9-tap matmul conv,
                                                       bf16, row-blocks
                                                       paired across the
                                                       two PE column tiles)
  4 coeff heads + attention head (conv+GAP)           (bf16 matmuls + accum)
  w_dyn synthesis (4 experts)                         (DVE chain -> fp8)
  main conv: out = conv(x, w_dyn*wm_coff), 3x3 pad 1  (fp8 DoubleRow: taps
                                                       packed in pairs into
                                                       the K dim, 5 passes)
The wm_coff modulation of x is folded into the conv1 weights (tower
side) and into the synthesized weights (main conv side), so the
full-res image is never rescaled. The fp8 weight quantization scale
(2^13) is folded into the per-expert coefficients and divided back out
in the PSUM-drain activation.
"""

import sys

if "/opt/trn_rl_repo" not in sys.path:
    sys.path.insert(0, "/opt/trn_rl_repo")

import numpy as np
import ml_dtypes
from contextlib import ExitStack

import concourse.bass as bass
import concourse.bacc as bacc
import concourse.tile as tile
from concourse import mybir
from concourse.bass import AP
from concourse.bass_utils import run_bass_kernel_spmd

F32 = mybir.dt.float32
BF16 = mybir.dt.bfloat16
F8 = mybir.dt.float8e4
U32 = mybir.dt.uint32
AF = mybir.ActivationFunctionType
OP = mybir.AluOpType
DR = mybir.MatmulPerfMode.DoubleRow

WDYN_SCALE = 8192.0   # wdyn absmax ~0.009 -> scale into e4m3's sweet spot


class Cfg:
    def __init__(self, BL=4, Cin=128, H=128, W=128, n_cores=8, main_fp8=True):
        self.BL, self.Cin, self.H, self.W, self.n_cores = BL, Cin, H, W, n_cores
        self.main_fp8 = main_fp8
        self.Cout = 64
        self.Hp, self.Wp = H + 2, W + 2
        self.NPAD = self.Hp * self.Wp
        self.NPAD1 = self.NPAD + 1   # +1 trailing zero for the dummy-tap AP
        # pooled size and tower conv output sizes
        self.P, self.PW = H // 2, W // 2
        self.c1h, self.c1w = self.P - 2, self.PW - 2
        self.c2h, self.c2w = (self.c1h - 3) // 2 + 1, (self.c1w - 3) // 2 + 1
        self.c3h, self.c3w = (self.c2h - 3) // 2 + 1, (self.c2w - 3) // 2 + 1
        self.hh, self.hw = (self.c3h - 3) // 2 + 1, (self.c3w - 3) // 2 + 1
        self.gapn = self.hh * self.hw
        # main conv row groups: pairs of row-groups share one PSUM bank
        self.RPG = 512 // W          # rows per row-group (N = RPG*W = 512)
        assert H % (2 * self.RPG) == 0
        self.NRG = H // self.RPG
        self.NPAIR = self.NRG // 2
        self.POUT = min(4, self.NPAIR)   # psum-pairs per output staging tile
        assert self.NPAIR % self.POUT == 0
        self.NOUT = self.NPAIR // self.POUT
        # ---- const blob layout (uint32 columns) ----
        # hot section first (needed immediately), cold (HD/EXP) last so the
        # first image's DMA can be queued between them
        o = 0

        def take(n):
            nonlocal o
            r = (o, o + n)
            o += n
            return r

        self.WM2 = take(self.Cin)                 # wm_w2T f32 [Cin, Cin]
        self.WM1 = take(self.Cin)                 # wm_w1T f32 [32, Cin]
        self.WMT = take(BL)                       # wm.T f32 [32, BL]
        self.AFW = take(4)                        # att_fw.T f32 [Cin, 4]
        self.W1 = take(9 * self.Cout // 2)        # w1T bf16 [Cin, 9*64]
        self.W2 = take(9 * self.Cout // 2)        # w2T bf16 [64, 9*64]
        self.W3 = take(9 * self.Cout // 2)        # w3T bf16 [64, 9*64]
        self.TB = take(3)                         # tower biases f32 [64, 3]
        self.HB = take(5)                         # head biases f32 [128, 5]
        self.AFB = take(4)                        # att_fb/gapn f32 [1, 4]
        self.WB1 = take(1)                        # wm_b1 f32 [128, 1]
        self.WB2 = take(1)                        # wm_b2 f32 [128, 1]
        self.ONE = take(self.Cin)                 # ones f32 [1, Cin]
        self.NHOT = o
        self.HD = take(5 * 9 * self.Cin // 2)     # headT bf16 [64, 5*9*128]
        self.EXP = take(4 * 9 * self.Cout // 2)   # expertT bf16 [Cin, 4*9*64]
        self.NBLOB = o


def _pack_f32(dst, col, arr):
    """Pack f32 array [r, c] into dst u32 blob at column col."""
    a = np.ascontiguousarray(arr, dtype=np.float32)
    dst[: a.shape[0], col : col + a.shape[1]] = a.view(np.uint32)


def _pack_bf16(dst, col, arr):
    a = np.ascontiguousarray(arr, dtype=ml_dtypes.bfloat16)
    u16 = a.view(np.uint16)
    u32 = (u16[:, 1::2].astype(np.uint32) << 16) | u16[:, 0::2].astype(np.uint32)
    dst[: a.shape[0], col : col + u32.shape[1]] = u32


def make_blob(cfg, wm_core, wm_w1, wm_b1, wm_w2, wm_b2, tr_w1, tr_b1, tr_w2,
              tr_b2, tr_w3, tr_b3, t1_w, t1_b, t2_w, t2_b, t3_w, t3_b, t4_w,
              t4_b, att_cw, att_cb, att_fw, att_fb, expert_w):
    """Pack all weights + this core's wm rows into one [128, NBLOB] u32 blob."""
    blob = np.zeros((128, cfg.NBLOB), np.uint32)
    # expertT[i, e, kh, kw, o] from expert_w[0][e, o, i, kh, kw]
    expT = expert_w[0].transpose(2, 0, 3, 4, 1).reshape(cfg.Cin, -1)
    _pack_bf16(blob, cfg.EXP[0], expT)
    _pack_f32(blob, cfg.WM2[0], wm_w2.T)
    _pack_f32(blob, cfg.WM1[0], wm_w1.T)
    _pack_f32(blob, cfg.WMT[0], wm_core.T)
    _pack_f32(blob, cfg.AFW[0], att_fw.T)
    # conv weights [o, i, kh, kw] -> [i, kh, kw, o]
    _pack_bf16(blob, cfg.W1[0], tr_w1.transpose(1, 2, 3, 0).reshape(cfg.Cin, -1))
    _pack_bf16(blob, cfg.W2[0], tr_w2.transpose(1, 2, 3, 0).reshape(64, -1))
    _pack_bf16(blob, cfg.W3[0], tr_w3.transpose(1, 2, 3, 0).reshape(64, -1))
    hd = np.concatenate(
        [w.transpose(1, 2, 3, 0).reshape(64, -1) for w in (t1_w, t2_w, t3_w, t4_w, att_cw)],
        axis=1,
    )
    _pack_bf16(blob, cfg.HD[0], hd)
    _pack_f32(blob, cfg.TB[0], np.stack([tr_b1, tr_b2, tr_b3], 1))
    _pack_f32(blob, cfg.HB[0], np.stack([t1_b, t2_b, t3_b, t4_b, att_cb], 1))
    _pack_f32(blob, cfg.AFB[0], (att_fb / cfg.gapn)[None, :])
    _pack_f32(blob, cfg.WB1[0], wm_b1[:, None])
    _pack_f32(blob, cfg.WB2[0], wm_b2[:, None])
    _pack_f32(blob, cfg.ONE[0], np.ones((1, cfg.Cin), np.float32))
    return blob


def build_nc(cfg):
    nc = bacc.Bacc()
    Cin, Cout, H, W = cfg.Cin, cfg.Cout, cfg.H, cfg.W
    XDT = F8 if cfg.main_fp8 else BF16
    xin = nc.declare_dram_parameter("x", [cfg.BL, Cin, cfg.NPAD1], XDT,
                                    isOutput=False)
    wblob = nc.declare_dram_parameter("wblob", [128, cfg.NBLOB], U32,
                                      isOutput=False)
    y = nc.declare_dram_parameter("y", [cfg.BL, Cout, H, W], F32, isOutput=True)

    with tile.TileContext(nc) as tc, ExitStack() as ctx:
        cpool = ctx.enter_context(tc.tile_pool(name="consts", bufs=1))
        xpool = ctx.enter_context(tc.tile_pool(name="xpad", bufs=1))
        dpool = ctx.enter_context(tc.tile_pool(name="data", bufs=1))
        spool = ctx.enter_context(tc.tile_pool(name="smalls", bufs=2))
        ypool = ctx.enter_context(tc.tile_pool(name="synth", bufs=2))
        wpool = ctx.enter_context(tc.tile_pool(name="wdyn", bufs=2))
        opool = ctx.enter_context(tc.tile_pool(name="outsb", bufs=2))
        mpsum = ctx.enter_context(tc.tile_pool(name="mpsum", bufs=4, space="PSUM"))
        tpsum = ctx.enter_context(tc.tile_pool(name="tpsum", bufs=2, space="PSUM"))
        hpsum = ctx.enter_context(tc.tile_pool(name="hpsum", bufs=2, space="PSUM"))

        blob = cpool.tile([128, cfg.NBLOB], U32)
        # hot weights first; HD/EXP queued after image 0 (see pipeline below)
        nc.gpsimd.dma_start(blob[:, 0 : cfg.NHOT], wblob[:, 0 : cfg.NHOT])

        def bl(rng, nrows=128, dt=F32):
            ap = blob[0:nrows, rng[0]: rng[1]]
            return ap.bitcast(dt)

        expT = bl(cfg.EXP, 128, BF16)
        wm_w2T = bl(cfg.WM2)
        wm_w1T = bl(cfg.WM1, 32)
        wmT = bl(cfg.WMT, 32)
        att_fwT = bl(cfg.AFW)
        w1T = bl(cfg.W1, 128, BF16)
        w2T = bl(cfg.W2, 64, BF16)
        w3T = bl(cfg.W3, 64, BF16)
        headT = bl(cfg.HD, 64, BF16)
        tr_b = bl(cfg.TB, 64)
        head_b = bl(cfg.HB)
        att_fb = bl(cfg.AFB, 1)
        wm_b1 = bl(cfg.WB1)
        wm_b2 = bl(cfg.WB2)
        ones_row = bl(cfg.ONE, 1)
        # wm-embedding scratch (written once, read per-sample)
        wmx = cpool.tile([128, 4 * cfg.BL], F32)
        hT = wmx[:, 0 : cfg.BL]
        wmc = wmx[:, cfg.BL : 2 * cfg.BL]
        wq = wmx[:, 2 * cfg.BL : 3 * cfg.BL]
        wmh = wmx[:, 3 * cfg.BL : 4 * cfg.BL]

        nxp = 3
        # padded-image buffers (borders arrive pre-zeroed from the host)
        xpads = [
            xpool.tile([128, cfg.NPAD1], XDT, tag=f"xp{i}", name=f"xp{i}")
            for i in range(nxp)
        ]
        xvs = [xp[:, 0 : cfg.NPAD].rearrange("p (r c) -> p r c", c=cfg.Wp)
               for xp in xpads]

        n_t1, n_t2, n_t3 = cfg.c1h * cfg.c1w, cfg.c2h * cfg.c2w, cfg.c3h * cfg.c3w
        tower = dpool.tile([64, n_t1 + n_t2 + n_t3], BF16, tag="tower")
        t1v = tower[:, 0:n_t1].rearrange("p (r c) -> p r c", c=cfg.c1w)
        t2v = tower[:, n_t1 : n_t1 + n_t2].rearrange("p (r c) -> p r c", c=cfg.c2w)
        t3v = tower[:, n_t1 + n_t2 :].rearrange("p (r c) -> p r c", c=cfg.c3w)

        def lrelu_inplace(ap, alpha):
            nc.vector.scalar_tensor_tensor(
                ap, ap, float(alpha), ap, op0=OP.mult, op1=OP.max
            )

        # ---- wm embedding -> wm_coff.T [Cin, BL] (once, all samples) ----
        ps = hpsum.tile([128, cfg.BL], F32, tag="hps")
        nc.tensor.matmul(ps[:], wm_w1T, wmT, start=True, stop=True)
        nc.scalar.activation(hT, ps[:], AF.Identity, bias=wm_b1)
        lrelu_inplace(hT, 0.2)
        ps = hpsum.tile([128, cfg.BL], F32, tag="hps")
        nc.tensor.matmul(ps[:], wm_w2T, hT, start=True, stop=True)
        nc.scalar.activation(wmc, ps[:], AF.Identity, bias=wm_b2)
        nc.vector.tensor_scalar_mul(wq, wmc, 0.25)
        nc.vector.tensor_scalar_mul(wmh, wmc, WDYN_SCALE if cfg.main_fp8
                                    else 1.0)

        prows = max(1, 512 // cfg.PW)   # pooled rows per pooling chunk
        assert cfg.P % prows == 0

        t0s, wdyns, w1ss = {}, {}, {}

        # ---------- per-sample stage emitters ----------
        def emit_dma(s):
            # chunked so pooling/conv can start before the full image lands
            xp = xpads[s % nxp]
            step = (cfg.Hp + 3) // 4 * cfg.Wp
            for c0 in range(0, cfg.NPAD1, step):
                c1 = min(cfg.NPAD1, c0 + step)
                nc.gpsimd.dma_start(xp[:, c0:c1], xin[s, :, c0:c1])

        def emit_pool(s):
            # per-sample conv1 weights with wm_coff/4 folded in; the pooled
            # image itself stays an unscaled 2x2 sum (saves a full-image
            # rescale pass through SBUF)
            w1s = ypool.tile([128, 9 * 64], BF16, tag="w1s", name=f"w1s_{s}")
            w1ss[s] = w1s
            nc.vector.tensor_scalar_mul(w1s[:], w1T, wq[:, s : s + 1])
            t0 = dpool.tile([128, cfg.P * cfg.PW], BF16, tag="t0", bufs=2,
                            name=f"t0_{s}")
            t0s[s] = t0
            xvp = xvs[s % nxp]
            for q in range(cfg.P // prows):
                rows = xvp[:, 1 + 2 * prows * q : 1 + 2 * prows * (q + 1),
                           1 : cfg.W + 1]
                blk = rows.rearrange("p (y a) (x b) -> p y x a b", a=2, b=2)
                sc = t0[:, prows * cfg.PW * q : prows * cfg.PW * (q + 1)]
                sc = sc.rearrange("p (y x) -> p y x", x=cfg.PW)
                with nc.allow_low_precision(reason="2x2 pool sum in bf16"):
                    nc.vector.tensor_reduce(sc, blk,
                                            axis=mybir.AxisListType.XY,
                                            op=OP.add)

        def emit_conv1(s):
            t0v = t0s.pop(s)[:].rearrange("p (r c) -> p r c", c=cfg.PW)
            w1s = w1ss.pop(s)
            # row-blocks ride the two PE column tiles in pairs
            pairs, y0 = [], 0
            while y0 < cfg.c1h:
                nb = min(8, (cfg.c1h - y0 + 1) // 2)
                pairs.append((y0, nb))
                y0 += 2 * nb
            for y0, nb in pairs:
                ps = tpsum.tile([128, nb * cfg.c1w], F32, tag="tps")
                for ky in range(3):
                    for kx in range(3):
                        wtap = w1s[:, (ky * 3 + kx) * 64 : (ky * 3 + kx + 1) * 64]
                        st = ky == 0 and kx == 0
                        sp = ky == 2 and kx == 2
                        for hf in range(2):
                            ya = y0 + hf * nb
                            nc.tensor.matmul(
                                ps[hf * 64 : hf * 64 + 64, :],
                                wtap,
                                t0v[:, ya + ky : ya + ky + nb,
                                    kx : kx + cfg.c1w],
                                start=st, stop=sp,
                            )
                for hf in range(2):
                    dst = t1v[:, y0 + hf * nb : y0 + (hf + 1) * nb, :]
                    nc.scalar.activation(dst, ps[hf * 64 : hf * 64 + 64, :],
                                         AF.Identity, bias=tr_b[:, 0:1])
                    lrelu_inplace(dst, 0.01)

        def emit_conv23(s):
            nb = cfg.c2h // 2
            ps = tpsum.tile([128, nb * cfg.c2w], F32, tag="tps")
            for ky in range(3):
                for kx in range(3):
                    wtap = w2T[:, (ky * 3 + kx) * 64 : (ky * 3 + kx + 1) * 64]
                    st = ky == 0 and kx == 0
                    sp = ky == 2 and kx == 2
                    for hf in range(2):
                        ya = hf * nb
                        rhs = t1v[:, 2 * ya + ky : 2 * ya + ky + 2 * nb : 2,
                                  kx : kx + 2 * cfg.c2w - 1 : 2]
                        nc.tensor.matmul(ps[hf * 64 : hf * 64 + 64, :],
                                         wtap, rhs, start=st, stop=sp)
            for hf in range(2):
                dst = t2v[:, hf * nb : (hf + 1) * nb, :]
                nc.scalar.activation(dst, ps[hf * 64 : hf * 64 + 64, :],
                                     AF.Identity, bias=tr_b[:, 1:2])
                lrelu_inplace(dst, 0.01)

            nb = cfg.c3h // 2
            ps = tpsum.tile([128, nb * cfg.c3w], F32, tag="tps")
            for ky in range(3):
                for kx in range(3):
                    wtap = w3T[:, (ky * 3 + kx) * 64 : (ky * 3 + kx + 1) * 64]
                    st = ky == 0 and kx == 0
                    sp = ky == 2 and kx == 2
                    for hf in range(2):
                        ya = hf * nb
                        rhs = t2v[:, 2 * ya + ky : 2 * ya + ky + 2 * nb : 2,
                                  kx : kx + 2 * cfg.c3w - 1 : 2]
                        nc.tensor.matmul(ps[hf * 64 : hf * 64 + 64, :],
                                         wtap, rhs, start=st, stop=sp)
            for hf in range(2):
                dst = t3v[:, hf * nb : (hf + 1) * nb, :]
                nc.scalar.activation(dst, ps[hf * 64 : hf * 64 + 64, :],
                                     AF.Identity, bias=tr_b[:, 2:3])
                lrelu_inplace(dst, 0.01)

        def emit_heads_att_synth(s):
            sm = spool.tile([128, 64], F32, tag="sm", name=f"sm_{s}")
            a_sb = sm[:, 0:1]
            att_row = sm[0:1, 4:8]
            att_bc = sm[:, 8:12]
            cc = sm[:, 12:16]
            gap = sm[:, 16:24]
            hscr = sm[:, 24:42].bitcast(BF16)[:, 0 : cfg.gapn]
            for h in range(5):
                ps = hpsum.tile([128, cfg.gapn], F32, tag="hps")
                for ky in range(3):
                    for kx in range(3):
                        rhs = t3v[:, ky : ky + 2 * cfg.hh - 1 : 2,
                                  kx : kx + 2 * cfg.hw - 1 : 2]
                        idx = h * 9 + ky * 3 + kx
                        nc.tensor.matmul(
                            ps[:],
                            headT[:, idx * 128 : (idx + 1) * 128],
                            rhs,
                            start=(ky == 0 and kx == 0),
                            stop=(ky == 2 and kx == 2),
                        )
                nc.scalar.activation(
                    hscr, ps[:], AF.Identity, bias=head_b[:, h : h + 1],
                    accum_out=gap[:, h : h + 1],
                )

            # attention: a = lrelu(gap4/gapn); att = (a@att_fwT + fb)/gapn
            nc.scalar.activation(a_sb, gap[:, 4:5], AF.Copy, scale=1.0 / cfg.gapn)
            lrelu_inplace(a_sb, 0.01)
            ps = hpsum.tile([1, 4], F32, tag="hps")
            nc.tensor.matmul(ps[:], a_sb, att_fwT, start=True, stop=True)
            nc.vector.scalar_tensor_tensor(
                att_row, ps[:], 1.0 / cfg.gapn, att_fb, op0=OP.mult, op1=OP.add
            )
            ps = hpsum.tile([128, 4], F32, tag="hps")
            nc.tensor.matmul(ps[:], ones_row, att_row, start=True, stop=True)
            nc.scalar.activation(att_bc, ps[:], AF.Copy)
            nc.vector.tensor_mul(cc, att_bc, gap[:, 0:4])
            # fold wm_coff (and the fp8 weight scale) into the coefficients
            nc.vector.tensor_scalar_mul(cc, cc, wmh[:, s : s + 1])

            # synthesize w_dynT[i, (kh kw o)]
            A = ypool.tile([128, 9 * 64], F32, tag="synA", name=f"synA_{s}")
            Bt = ypool.tile([128, 9 * 64], F32, tag="synB", name=f"synB_{s}")
            if cfg.main_fp8:
                wdyn = wpool.tile([128, 10 * 64], F8, tag="wdyn",
                                  name=f"wdyn_{s}")
                nc.gpsimd.memset(wdyn[:, 9 * 64 : 10 * 64], 0)
                wtail = wdyn[:, 0 : 9 * 64]
            else:
                wdyn = wpool.tile([128, 9 * 64], BF16, tag="wdyn",
                                  name=f"wdyn_{s}")
                wtail = wdyn[:]
            wdyns[s] = wdyn
            nc.vector.tensor_scalar_mul(A[:], expT[:, 0:576], cc[:, 0:1])
            nc.vector.scalar_tensor_tensor(
                Bt[:], expT[:, 576:1152], cc[:, 1:2], A[:], op0=OP.mult,
                op1=OP.add,
            )
            nc.vector.scalar_tensor_tensor(
                A[:], expT[:, 1152:1728], cc[:, 2:3], Bt[:], op0=OP.mult,
                op1=OP.add,
            )
            with nc.allow_low_precision(reason="wdyn is consumed in fp8/bf16"):
                nc.vector.scalar_tensor_tensor(
                    wtail, expT[:, 1728:2304], cc[:, 3:4], A[:], op0=OP.mult,
                    op1=OP.add,
                )

        # main conv taps packed two-per-pass for fp8 DoubleRow: each entry is
        # (ky0, kx0, elem offset of the second tap); the 5th pass pairs tap
        # (2,2) with an all-zero weight block (reads one elem past the row,
        # which is why the image buffer carries one trailing zero byte)
        TAP_PAIRS = [(0, 0, 1), (0, 2, cfg.Wp - 2), (1, 1, 1), (2, 0, 1),
                     (2, 2, 1)]

        def emit_main_group(s, q):
            xp = xpads[s % nxp]
            flat = xp[:]
            wdyn = wdyns[s]
            out_t = opool.tile([128, cfg.POUT * 512], F32, tag="outsb",
                               name=f"out_{s}_{q}")
            oscale = 1.0 / WDYN_SCALE if cfg.main_fp8 else 1.0
            for j in range(cfg.POUT):
                pair = q * cfg.POUT + j
                ps = mpsum.tile([128, 512], F32, tag="mps")
                if cfg.main_fp8:
                    for p, (ky0, kx0, off) in enumerate(TAP_PAIRS):
                        lhs = wdyn[:, p * 128 : (p + 1) * 128].rearrange(
                            "p (t o) -> p t o", t=2
                        )
                        for half in range(2):
                            y0 = (2 * pair + half) * cfg.RPG
                            base = (y0 + ky0) * cfg.Wp + kx0
                            rhs = AP(
                                flat.tensor, flat.offset + base,
                                [list(flat.ap[0]), [off, 2],
                                 [cfg.Wp, cfg.RPG], [1, cfg.W]],
                            )
                            nc.tensor.matmul(
                                ps[half * 64 : half * 64 + 64, :],
                                lhs, rhs,
                                start=(p == 0), stop=(p == 4),
                                perf_mode=DR,
                            )
                else:
                    xv = xvs[s % nxp]
                    for ky in range(3):
                        for kx in range(3):
                            for half in range(2):
                                y0 = (2 * pair + half) * cfg.RPG
                                nc.tensor.matmul(
                                    ps[half * 64 : half * 64 + 64, :],
                                    wdyn[:, (ky * 3 + kx) * 64 : (ky * 3 + kx + 1) * 64],
                                    xv[:, y0 + ky : y0 + ky + cfg.RPG,
                                       kx : kx + cfg.W],
                                    start=(ky == 0 and kx == 0),
                                    stop=(ky == 2 and kx == 2),
                                )
                nc.scalar.activation(
                    out_t[:, j * 512 : (j + 1) * 512], ps[:], AF.Copy,
                    scale=oscale,
                )
                if s == cfg.BL - 1 and q == cfg.NOUT - 1:
                    # drain the very last group per pair so the kernel
                    # tail is one small DMA, not a whole group's worth
                    yvj = y[s].rearrange("c (j r) x -> c j r x",
                                         r=2 * cfg.RPG)
                    for hf in range(2):
                        nc.gpsimd.dma_start(
                            yvj[:, q * cfg.POUT + j,
                                hf * cfg.RPG : (hf + 1) * cfg.RPG, :],
                            out_t[hf * 64 : hf * 64 + 64,
                                  j * 512 : (j + 1) * 512],
                        )
            if not (s == cfg.BL - 1 and q == cfg.NOUT - 1):
                yv = y[s].rearrange("c (j r) x -> c j r x", r=2 * cfg.RPG)
                jj = q * cfg.POUT
                for hf in range(2):
                    dst = yv[:, jj : jj + cfg.POUT,
                             hf * cfg.RPG : (hf + 1) * cfg.RPG, :]
                    nc.gpsimd.dma_start(dst, out_t[hf * 64 : hf * 64 + 64, :])
            if q == cfg.NOUT - 1:
                wdyns.pop(s)

        # ---------- software pipeline ----------
        # prologue: samples 0 (and 1) fully up to synth before main(0)
        emit_dma(0)
        # cold weights (heads, experts) land after image 0
        nc.gpsimd.dma_start(blob[:, cfg.NHOT :], wblob[:, cfg.NHOT :])
        emit_pool(0)
        if cfg.BL > 1:
            emit_dma(1)
        emit_conv1(0)
        emit_conv23(0)
        emit_heads_att_synth(0)
        if cfg.BL > 1:
            emit_pool(1)

        # stage k of sample s+1 (or s+2 for dma/pool) after main group q=k
        def stage_after(s, q):
            if q == min(0, cfg.NOUT - 1):
                if s + 2 < cfg.BL and nxp >= 3:
                    emit_dma(s + 2)
                if s + 1 < cfg.BL:
                    emit_conv1(s + 1)
            if q == min(1, cfg.NOUT - 1):
                if s + 1 < cfg.BL:
                    emit_conv23(s + 1)
            if q == min(2, cfg.NOUT - 1):
                if s + 1 < cfg.BL:
                    emit_heads_att_synth(s + 1)
            if q == cfg.NOUT - 1:
                if s + 2 < cfg.BL and nxp < 3:
                    emit_dma(s + 2)
                if s + 2 < cfg.BL:
                    emit_pool(s + 2)

        for s in range(cfg.BL):
            for q in range(cfg.NOUT):
                emit_main_group(s, q)
                stage_after(s, q)

    return nc


_NC_CACHE = {}
TRACE = False       # set by test harness to collect an NTFF profile
TRACE_DIR = None    # where to leave the NTFF/perfetto artifacts
LAST_RESULT = None  # BassKernelResults of the most recent kernel() call


def _get_nc(cfg):
    key = (cfg.BL, cfg.Cin, cfg.H, cfg.W, cfg.main_fp8)
    if key not in _NC_CACHE:
        nc = build_nc(cfg)
        if not nc.is_finalized():
            nc.finalize()
        _NC_CACHE[key] = nc
    return _NC_CACHE[key]


def pad_images(cfg, x):
    """[n, Cin, H, W] -> zero-padded flat [n, Cin, NPAD1]."""
    n = x.shape[0]
    dt = ml_dtypes.float8_e4m3 if cfg.main_fp8 else ml_dtypes.bfloat16
    xp = np.zeros((n, cfg.Cin, cfg.NPAD1), dt)
    xpv = xp[:, :, : cfg.NPAD].reshape(n, cfg.Cin, cfg.Hp, cfg.Wp)
    xpv[:, :, 1 : cfg.H + 1, 1 : cfg.W + 1] = x.astype(dt)
    return xp


MAIN_FP8 = False   # main conv in fp8 DoubleRow vs bf16 column-tile pairs


def kernel(**inputs):
    x = np.asarray(inputs["x"], np.float32)
    B, Cin, H, W = x.shape
    cfg = Cfg(BL=B // 8, Cin=Cin, H=H, W=W, main_fp8=MAIN_FP8)
    nc = _get_nc(cfg)
    wnames = [
        "wm_w1", "wm_b1", "wm_w2", "wm_b2", "tr_w1", "tr_b1", "tr_w2", "tr_b2",
        "tr_w3", "tr_b3", "t1_w", "t1_b", "t2_w", "t2_b", "t3_w", "t3_b",
        "t4_w", "t4_b", "att_cw", "att_cb", "att_fw", "att_fb", "expert_w",
    ]
    ws = {k: np.asarray(inputs[k], np.float32) for k in wnames}
    wm = np.asarray(inputs["wm"], np.float32)
    in_maps = []
    for c in range(8):
        sl = slice(c * cfg.BL, (c + 1) * cfg.BL)
        blob = make_blob(cfg, wm[sl], **ws)
        in_maps.append({"x": pad_images(cfg, x[sl]), "wblob": blob})
    global LAST_RESULT
    kw = {"tmpdir": TRACE_DIR} if (TRACE and TRACE_DIR) else {}
    res = run_bass_kernel_spmd(nc, in_maps, list(range(8)), trace=TRACE, **kw)
    LAST_RESULT = res
    return np.concatenate([res.results[c]["y"] for c in range(8)], axis=0)



# revision 5
# speedup vs baseline: 1.1773x; 1.1773x over previous
"""Trainium2 Bass kernel for nn_CustomConvLayer (dynamic per-sample conv).

Sharding: pure data parallel over batch B=32 across 8 NeuronCores (4
samples per core). Small synthesis networks are replicated per core.

Per sample, on device:
  wm-embedding MLP -> per-channel modulation wm_coff  (tiny matmuls)
  t0 = 2x2 sum-pool(x)                                (DVE 2-pass adds)
  tower: 3 convs (lrelu) -> t3                        (9-tap matmul conv,
                                                       bf16, row-blocks
                                                       paired across the
                                                       two PE column tiles)
  4 coeff heads + attention head (conv+GAP)           (pre-summed s-vectors
                                                       + thin matmuls)
  w_dyn synthesis (4 experts)                         (DVE chain)
  main conv: out = conv(x, w_dyn*wm_coff), 3x3 pad 1  (bf16 col-tile pairs,
                                                       tap-major with
                                                       weight reuse)
The wm_coff modulation of x is folded into the conv1 weights (tower
side) and into the synthesized weights (main conv side), so the
full-res image is never rescaled.

Main-conv structure: per group of 4 psum pairs, the 9 taps are walked
tap-major so each tap's weights are loaded into the two PE column
tiles once and reused by 8 matmuls (ldweights=False) — the LDWEIGHTS
SBUF reads otherwise steal streaming port cycles.  GAP heads are
computed from 9 pre-summed per-tap vectors s[i,tap] (DVE window
reduces), turning 45 LDW-bound matmuls into 18 thin ones.
"""

import sys

if "/opt/trn_rl_repo" not in sys.path:
    sys.path.insert(0, "/opt/trn_rl_repo")

import numpy as np
import ml_dtypes
from contextlib import ExitStack

import concourse.bass as bass
import concourse.bacc as bacc
import concourse.tile as tile
from concourse import mybir
from concourse.bass import AP
from concourse.bass_utils import run_bass_kernel_spmd

F32 = mybir.dt.float32
BF16 = mybir.dt.bfloat16
U32 = mybir.dt.uint32
AF = mybir.ActivationFunctionType
OP = mybir.AluOpType

AMORTIZE_LDW = True   # reuse PE-loaded weights across matmuls (tap-major)
FUSED_LRELU = True    # leaky-relu folded into the PSUM-drain activation


class Cfg:
    def __init__(self, BL=4, Cin=128, H=128, W=128, n_cores=8):
        self.BL, self.Cin, self.H, self.W, self.n_cores = BL, Cin, H, W, n_cores
        self.Cout = 64
        self.Hp, self.Wp = H + 2, W + 2
        self.NPAD = self.Hp * self.Wp
        self.NPAD1 = self.NPAD + 1
        # pooled size and tower conv output sizes
        self.P, self.PW = H // 2, W // 2
        self.c1h, self.c1w = self.P - 2, self.PW - 2
        self.c2h, self.c2w = (self.c1h - 3) // 2 + 1, (self.c1w - 3) // 2 + 1
        self.c3h, self.c3w = (self.c2h - 3) // 2 + 1, (self.c2w - 3) // 2 + 1
        self.hh, self.hw = (self.c3h - 3) // 2 + 1, (self.c3w - 3) // 2 + 1
        self.gapn = self.hh * self.hw
        # main conv row groups: pairs of row-groups share one PSUM bank
        self.RPG = 512 // W          # rows per row-group (N = RPG*W = 512)
        assert H % (2 * self.RPG) == 0
        self.NRG = H // self.RPG
        self.NPAIR = self.NRG // 2
        self.POUT = min(4, self.NPAIR)   # psum-pairs per group
        assert self.NPAIR % self.POUT == 0
        self.NOUT = self.NPAIR // self.POUT
        # ---- const blob layout (uint32 columns) ----
        o = 0

        def take(n):
            nonlocal o
            r = (o, o + n)
            o += n
            return r

        self.WM2 = take(self.Cin)                 # wm_w2T f32 [Cin, Cin]
        self.WM1 = take(self.Cin)                 # wm_w1T f32 [32, Cin]
        self.WMT = take(BL)                       # wm.T f32 [32, BL]
        self.AFW = take(4)                        # att_fw.T f32 [Cin, 4]
        self.W1 = take(9 * self.Cout // 2)        # w1T bf16 [Cin, 9*64]
        self.W2 = take(9 * self.Cout // 2)        # w2T bf16 [64, 9*64]
        self.W3 = take(9 * self.Cout // 2)        # w3T bf16 [64, 9*64]
        self.TB = take(3)                         # tower biases f32 [64, 3]
        self.HB = take(5)                         # head biases f32 [128, 5]
        self.AFB = take(4)                        # att_fb f32 [1, 4]
        self.WB1 = take(1)                        # wm_b1 f32 [128, 1]
        self.WB2 = take(1)                        # wm_b2 f32 [128, 1]
        self.ONE = take(self.Cin)                 # ones f32 [1, Cin]
        self.ONEB = take(1)                       # ones bf16 [1, 2]
        self.NHOT = o
        self.HD = take(5 * 9 * self.Cin // 2)     # headT bf16 [64, 9*5*128]
        self.EXP = take(4 * 9 * self.Cout // 2)   # expertT bf16 [Cin, 4*9*64]
        self.NBLOB = o


def _pack_f32(dst, col, arr):
    a = np.ascontiguousarray(arr, dtype=np.float32)
    dst[: a.shape[0], col : col + a.shape[1]] = a.view(np.uint32)


def _pack_bf16(dst, col, arr):
    a = np.ascontiguousarray(arr, dtype=ml_dtypes.bfloat16)
    u16 = a.view(np.uint16)
    u32 = (u16[:, 1::2].astype(np.uint32) << 16) | u16[:, 0::2].astype(np.uint32)
    dst[: a.shape[0], col : col + u32.shape[1]] = u32


def make_blob(cfg, wm_core, wm_w1, wm_b1, wm_w2, wm_b2, tr_w1, tr_b1, tr_w2,
              tr_b2, tr_w3, tr_b3, t1_w, t1_b, t2_w, t2_b, t3_w, t3_b, t4_w,
              t4_b, att_cw, att_cb, att_fw, att_fb, expert_w):
    """Pack all weights + this core's wm rows into one [128, NBLOB] u32 blob."""
    blob = np.zeros((128, cfg.NBLOB), np.uint32)
    # expertT[i, e, kh, kw, o] from expert_w[0][e, o, i, kh, kw]
    expT = expert_w[0].transpose(2, 0, 3, 4, 1).reshape(cfg.Cin, -1)
    _pack_bf16(blob, cfg.EXP[0], expT)
    _pack_f32(blob, cfg.WM2[0], wm_w2.T)
    _pack_f32(blob, cfg.WM1[0], wm_w1.T)
    _pack_f32(blob, cfg.WMT[0], wm_core.T)
    _pack_f32(blob, cfg.AFW[0], att_fw.T)
    # conv weights [o, i, kh, kw] -> [i, kh, kw, o]
    _pack_bf16(blob, cfg.W1[0], tr_w1.transpose(1, 2, 3, 0).reshape(cfg.Cin, -1))
    _pack_bf16(blob, cfg.W2[0], tr_w2.transpose(1, 2, 3, 0).reshape(64, -1))
    _pack_bf16(blob, cfg.W3[0], tr_w3.transpose(1, 2, 3, 0).reshape(64, -1))
    # head weights [h, o, i, ky, kx] -> [i, (ky kx h o)]
    hd5 = np.stack([t1_w, t2_w, t3_w, t4_w, att_cw])      # [5,128,64,3,3]
    hd2 = hd5.transpose(2, 3, 4, 0, 1).reshape(64, -1)    # [64, 9*5*128]
    _pack_bf16(blob, cfg.HD[0], hd2)
    _pack_f32(blob, cfg.TB[0], np.stack([tr_b1, tr_b2, tr_b3], 1))
    _pack_f32(blob, cfg.HB[0], np.stack([t1_b, t2_b, t3_b, t4_b, att_cb], 1))
    _pack_f32(blob, cfg.AFB[0], att_fb[None, :])
    _pack_f32(blob, cfg.WB1[0], wm_b1[:, None])
    _pack_f32(blob, cfg.WB2[0], wm_b2[:, None])
    _pack_f32(blob, cfg.ONE[0], np.ones((1, cfg.Cin), np.float32))
    _pack_bf16(blob, cfg.ONEB[0], np.ones((1, 2), np.float32))
    return blob


def build_nc(cfg):
    nc = bacc.Bacc()
    Cin, Cout, H, W = cfg.Cin, cfg.Cout, cfg.H, cfg.W
    xin = nc.declare_dram_parameter("x", [cfg.BL, Cin, cfg.NPAD1], BF16,
                                    isOutput=False)
    wblob = nc.declare_dram_parameter("wblob", [128, cfg.NBLOB], U32,
                                      isOutput=False)
    y = nc.declare_dram_parameter("y", [cfg.BL, Cout, H, W], F32, isOutput=True)

    with tile.TileContext(nc) as tc, ExitStack() as ctx:
        cpool = ctx.enter_context(tc.tile_pool(name="consts", bufs=1))
        xpool = ctx.enter_context(tc.tile_pool(name="xpad", bufs=1))
        dpool = ctx.enter_context(tc.tile_pool(name="data", bufs=1))
        spool = ctx.enter_context(tc.tile_pool(name="smalls", bufs=2))
        ypool = ctx.enter_context(tc.tile_pool(name="synth", bufs=2))
        wpool = ctx.enter_context(tc.tile_pool(name="wdyn", bufs=2))
        opool = ctx.enter_context(tc.tile_pool(name="outsb", bufs=2))
        mpsum = ctx.enter_context(tc.tile_pool(name="mpsum", bufs=4, space="PSUM"))
        tpsum = ctx.enter_context(tc.tile_pool(name="tpsum", bufs=4, space="PSUM"))

        blob = cpool.tile([128, cfg.NBLOB], U32)
        # hot weights first; HD/EXP queued after image 0 (see pipeline below)
        nc.gpsimd.dma_start(blob[:, 0 : cfg.NHOT], wblob[:, 0 : cfg.NHOT])

        def bl(rng, nrows=128, dt=F32):
            ap = blob[0:nrows, rng[0]: rng[1]]
            return ap.bitcast(dt)

        expT = bl(cfg.EXP, 128, BF16)
        wm_w2T = bl(cfg.WM2)
        wm_w1T = bl(cfg.WM1, 32)
        wmT = bl(cfg.WMT, 32)
        att_fwT = bl(cfg.AFW)
        w1T = bl(cfg.W1, 128, BF16)
        w2T = bl(cfg.W2, 64, BF16)
        w3T = bl(cfg.W3, 64, BF16)
        headT = bl(cfg.HD, 64, BF16)
        tr_b = bl(cfg.TB, 64)
        head_b = bl(cfg.HB)
        att_fb = bl(cfg.AFB, 1)
        wm_b1 = bl(cfg.WB1)
        wm_b2 = bl(cfg.WB2)
        ones_row = bl(cfg.ONE, 1)
        one_bf = bl(cfg.ONEB, 1, BF16)
        # wm-embedding scratch (written once, read per-sample)
        wmx = cpool.tile([128, 4 * cfg.BL], F32)
        hT = wmx[:, 0 : cfg.BL]
        wmc = wmx[:, cfg.BL : 2 * cfg.BL]
        wq = wmx[:, 2 * cfg.BL : 3 * cfg.BL]
        wmh = wmx[:, 3 * cfg.BL : 4 * cfg.BL]

        nxp = 3
        xpads = [
            xpool.tile([128, cfg.NPAD1], BF16, tag=f"xp{i}", name=f"xp{i}")
            for i in range(nxp)
        ]
        xvs = [xp[:, 0 : cfg.NPAD].rearrange("p (r c) -> p r c", c=cfg.Wp)
               for xp in xpads]

        n_t1, n_t2, n_t3 = cfg.c1h * cfg.c1w, cfg.c2h * cfg.c2w, cfg.c3h * cfg.c3w
        tower = dpool.tile([64, n_t1 + n_t2 + n_t3], BF16, tag="tower")
        t1v = tower[:, 0:n_t1].rearrange("p (r c) -> p r c", c=cfg.c1w)
        t2v = tower[:, n_t1 : n_t1 + n_t2].rearrange("p (r c) -> p r c", c=cfg.c2w)
        t3v = tower[:, n_t1 + n_t2 :].rearrange("p (r c) -> p r c", c=cfg.c3w)

        def lrelu_inplace(ap, alpha):
            nc.vector.scalar_tensor_tensor(
                ap, ap, float(alpha), ap, op0=OP.mult, op1=OP.max
            )

        def mm(ps, lhsT, rhs, start, stop, reuse_w=False):
            mi = nc.tensor.matmul(ps, lhsT, rhs, start=start, stop=stop)
            if reuse_w and AMORTIZE_LDW:
                mi.ins.ldweights = False
            return mi

        # ---- wm embedding -> wm_coff.T [Cin, BL] (once, all samples) ----
        ps = tpsum.tile([128, cfg.BL], F32, tag="tps")
        nc.tensor.matmul(ps[:], wm_w1T, wmT, start=True, stop=True)
        # AF.Lrelu has a fixed 0.01 slope (alpha is ignored), so the 0.2-slope
        # leaky relu must stay an explicit DVE op
        nc.scalar.activation(hT, ps[:], AF.Identity, bias=wm_b1)
        lrelu_inplace(hT, 0.2)
        ps = tpsum.tile([128, cfg.BL], F32, tag="tps")
        nc.tensor.matmul(ps[:], wm_w2T, hT, start=True, stop=True)
        nc.scalar.activation(wmc, ps[:], AF.Identity, bias=wm_b2)
        nc.vector.tensor_scalar_mul(wq, wmc, 0.25)
        nc.vector.tensor_scalar_mul(wmh, wmc, 1.0)

        t0s, wdyns, w1ss = {}, {}, {}

        # ---------- per-sample stage emitters ----------
        def emit_dma(s):
            # chunked so pooling/conv can start before the full image lands
            xp = xpads[s % nxp]
            step = (cfg.Hp + 3) // 4 * cfg.Wp
            for c0 in range(0, cfg.NPAD1, step):
                c1 = min(cfg.NPAD1, c0 + step)
                nc.gpsimd.dma_start(xp[:, c0:c1], xin[s, :, c0:c1])

        def emit_pool(s):
            # per-sample conv1 weights with wm_coff/4 folded in; the pooled
            # image itself stays an unscaled 2x2 sum.  Pool = two DVE
            # tensor-add passes (row pairs at 2x bf16 rate, then column
            # pairs) — much cheaper than a windowed tensor_reduce.
            w1s = ypool.tile([128, 9 * 64], BF16, tag="w1s", name=f"w1s_{s}")
            w1ss[s] = w1s
            nc.vector.tensor_scalar_mul(w1s[:], w1T, wq[:, s : s + 1])
            t0 = dpool.tile([128, cfg.P * cfg.PW], BF16, tag="t0", bufs=2,
                            name=f"t0_{s}")
            t0s[s] = t0
            xvp = xvs[s % nxp]
            t0r = t0[:].rearrange("p (r c) -> p r c", c=cfg.PW)
            nch = 8
            prows = cfg.P // nch            # pooled rows per chunk
            for q in range(nch):
                r0 = prows * q
                scr = spool.tile([128, prows * cfg.W], BF16, tag="pscr",
                                 name=f"pscr_{s}_{q}")
                sv = scr[:].rearrange("p (r c) -> p r c", c=cfg.W)
                in0 = xvp[:, 1 + 2 * r0 : 1 + 2 * (r0 + prows) : 2, 1 : cfg.W + 1]
                in1 = xvp[:, 2 + 2 * r0 : 2 + 2 * (r0 + prows) : 2, 1 : cfg.W + 1]
                with nc.allow_low_precision(reason="2x2 pool sum in bf16"):
                    nc.vector.tensor_add(sv, in0, in1)
                    nc.vector.tensor_add(
                        t0r[:, r0 : r0 + prows, :],
                        sv[:, :, 0 : cfg.W : 2],
                        sv[:, :, 1 : cfg.W : 2],
                    )

        def emit_conv1(s, tap_major):
            t0v = t0s.pop(s)[:].rearrange("p (r c) -> p r c", c=cfg.PW)
            w1s = w1ss.pop(s)
            pairs, y0 = [], 0
            while y0 < cfg.c1h:
                nb = min(8, (cfg.c1h - y0 + 1) // 2)
                pairs.append((y0, nb))
                y0 += 2 * nb

            def c1_drain(ps, y0, nb):
                for hf in range(2):
                    dst = t1v[:, y0 + hf * nb : y0 + (hf + 1) * nb, :]
                    src = ps[hf * 64 : hf * 64 + 64, 0 : nb * cfg.c1w]
                    if FUSED_LRELU:
                        nc.scalar.activation(dst, src, AF.Lrelu,
                                             bias=tr_b[:, 0:1], alpha=0.01)
                    else:
                        nc.scalar.activation(dst, src, AF.Identity,
                                             bias=tr_b[:, 0:1])
                        lrelu_inplace(dst, 0.01)

            if not tap_major:
                # latency-optimal for sample 0: each row-block pair finishes
                # as soon as its pooled rows land
                for y0, nb in pairs:
                    ps = tpsum.tile([128, nb * cfg.c1w], F32, tag="tps")
                    for ky in range(3):
                        for kx in range(3):
                            wtap = w1s[:, (ky * 3 + kx) * 64 : (ky * 3 + kx + 1) * 64]
                            st = ky == 0 and kx == 0
                            sp = ky == 2 and kx == 2
                            for hf in range(2):
                                ya = y0 + hf * nb
                                nc.tensor.matmul(
                                    ps[hf * 64 : hf * 64 + 64, :], wtap,
                                    t0v[:, ya + ky : ya + ky + nb,
                                        kx : kx + cfg.c1w],
                                    start=st, stop=sp,
                                )
                    c1_drain(ps, y0, nb)
                return
            # tap-major across all 4 block-pairs: each tap's weights are
            # loaded into the two column tiles once and reused 8x
            pss = [tpsum.tile([128, nb * cfg.c1w], F32, tag="tps",
                              name=f"c1ps_{s}_{bi}")
                   for bi, (_, nb) in enumerate(pairs)]
            for ky in range(3):
                for kx in range(3):
                    wtap = w1s[:, (ky * 3 + kx) * 64 : (ky * 3 + kx + 1) * 64]
                    st = ky == 0 and kx == 0
                    sp = ky == 2 and kx == 2
                    for bi, (y0, nb) in enumerate(pairs):
                        for hf in range(2):
                            ya = y0 + hf * nb
                            mm(pss[bi][hf * 64 : hf * 64 + 64, :], wtap,
                               t0v[:, ya + ky : ya + ky + nb, kx : kx + cfg.c1w],
                               start=st, stop=sp, reuse_w=(bi > 0))
            for bi, (y0, nb) in enumerate(pairs):
                c1_drain(pss[bi], y0, nb)

        def emit_conv23(s):
            nb = cfg.c2h // 2
            ps = tpsum.tile([128, nb * cfg.c2w], F32, tag="tps")
            for ky in range(3):
                for kx in range(3):
                    wtap = w2T[:, (ky * 3 + kx) * 64 : (ky * 3 + kx + 1) * 64]
                    st = ky == 0 and kx == 0
                    sp = ky == 2 and kx == 2
                    for hf in range(2):
                        ya = hf * nb
                        rhs = t1v[:, 2 * ya + ky : 2 * ya + ky + 2 * nb : 2,
                                  kx : kx + 2 * cfg.c2w - 1 : 2]
                        nc.tensor.matmul(ps[hf * 64 : hf * 64 + 64, :],
                                         wtap, rhs, start=st, stop=sp)
            for hf in range(2):
                dst = t2v[:, hf * nb : (hf + 1) * nb, :]
                src = ps[hf * 64 : hf * 64 + 64, :]
                if FUSED_LRELU:
                    nc.scalar.activation(dst, src, AF.Lrelu,
                                         bias=tr_b[:, 1:2], alpha=0.01)
                else:
                    nc.scalar.activation(dst, src, AF.Identity, bias=tr_b[:, 1:2])
                    lrelu_inplace(dst, 0.01)

            nb = cfg.c3h // 2
            ps = tpsum.tile([128, nb * cfg.c3w], F32, tag="tps")
            for ky in range(3):
                for kx in range(3):
                    wtap = w3T[:, (ky * 3 + kx) * 64 : (ky * 3 + kx + 1) * 64]
                    st = ky == 0 and kx == 0
                    sp = ky == 2 and kx == 2
                    for hf in range(2):
                        ya = hf * nb
                        rhs = t2v[:, 2 * ya + ky : 2 * ya + ky + 2 * nb : 2,
                                  kx : kx + 2 * cfg.c3w - 1 : 2]
                        nc.tensor.matmul(ps[hf * 64 : hf * 64 + 64, :],
                                         wtap, rhs, start=st, stop=sp)
            for hf in range(2):
                dst = t3v[:, hf * nb : (hf + 1) * nb, :]
                src = ps[hf * 64 : hf * 64 + 64, :]
                if FUSED_LRELU:
                    nc.scalar.activation(dst, src, AF.Lrelu,
                                         bias=tr_b[:, 2:3], alpha=0.01)
                else:
                    nc.scalar.activation(dst, src, AF.Identity, bias=tr_b[:, 2:3])
                    lrelu_inplace(dst, 0.01)

        def emit_heads_att_synth(s):
            # GAP(conv(t3,W)) = W . s/36 + b with s[i,tap] = sum over the
            # 6x6 output grid of t3 windows: 9 DVE reduces + thin matmuls
            # instead of 45 LDW-bound wide-weight matmuls.
            sm = spool.tile([128, 32], F32, tag="sm", name=f"sm_{s}")
            a_sb = sm[:, 0:1]
            att_row = sm[0:1, 4:8]
            att_bc = sm[:, 8:12]
            cc = sm[:, 12:16]
            gap = sm[:, 16:21]
            s_bf = spool.tile([64, 16], BF16, tag="sbf", name=f"sbf_{s}")
            hv = spool.tile([1, 5 * 128], BF16, tag="hv", name=f"hv_{s}")
            for t in range(9):
                ky, kx = t // 3, t % 3
                src = t3v[0:64, ky : ky + 2 * cfg.hh - 1 : 2,
                          kx : kx + 2 * cfg.hw - 1 : 2]
                with nc.allow_low_precision(reason="head window sums in bf16"):
                    nc.vector.tensor_reduce(s_bf[:, t : t + 1], src,
                                            axis=mybir.AxisListType.XY,
                                            op=OP.add)
            # hv[(h o)] = sum_t s[:,t] . headT[:, t*640 : (t+1)*640]
            ph = tpsum.tile([1, 512], F32, tag="tps")
            ph2 = tpsum.tile([1, 128], F32, tag="tps")
            for t in range(9):
                lhs = s_bf[:, t : t + 1]
                nc.tensor.matmul(ph[:], lhs,
                                 headT[:, t * 640 : t * 640 + 512],
                                 start=(t == 0), stop=(t == 8))
                nc.tensor.matmul(ph2[:], lhs,
                                 headT[:, t * 640 + 512 : (t + 1) * 640],
                                 start=(t == 0), stop=(t == 8))
            with nc.allow_low_precision(reason="head gaps via bf16"):
                nc.scalar.activation(hv[0:1, 0:512], ph[:], AF.Copy)
                nc.scalar.activation(hv[0:1, 512:640], ph2[:], AF.Copy)
            # transpose the 5 GAP vectors onto partitions: [1,128] @ [1,1]
            gps = tpsum.tile([128, 8], F32, tag="tps")
            for h in range(5):
                nc.tensor.matmul(gps[:, h : h + 1],
                                 hv[0:1, h * 128 : (h + 1) * 128],
                                 one_bf[0:1, 0:1],
                                 start=(h == 0), stop=(h == 4))
            for h in range(5):
                nc.scalar.activation(gap[:, h : h + 1], gps[:, h : h + 1],
                                     AF.Identity, bias=head_b[:, h : h + 1],
                                     scale=1.0 / cfg.gapn)

            # attention: a = lrelu(gap[:,4]); att = a@att_fwT + att_fb
            nc.vector.scalar_tensor_tensor(a_sb, gap[:, 4:5], 0.01,
                                           gap[:, 4:5], op0=OP.mult, op1=OP.max)
            ps = tpsum.tile([1, 4], F32, tag="tps")
            nc.tensor.matmul(ps[:], a_sb, att_fwT, start=True, stop=True)
            nc.vector.tensor_add(att_row, ps[:], att_fb)
            ps = tpsum.tile([128, 4], F32, tag="tps")
            nc.tensor.matmul(ps[:], ones_row, att_row, start=True, stop=True)
            nc.scalar.activation(att_bc, ps[:], AF.Copy)
            nc.vector.tensor_mul(cc, att_bc, gap[:, 0:4])
            # fold wm_coff into the coefficients
            nc.vector.tensor_scalar_mul(cc, cc, wmh[:, s : s + 1])

            # synthesize w_dynT[i, (kh kw o)]
            A = ypool.tile([128, 9 * 64], F32, tag="synA", name=f"synA_{s}")
            Bt = ypool.tile([128, 9 * 64], F32, tag="synB", name=f"synB_{s}")
            wdyn = wpool.tile([128, 9 * 64], BF16, tag="wdyn", name=f"wdyn_{s}")
            wdyns[s] = wdyn
            nc.vector.tensor_scalar_mul(A[:], expT[:, 0:576], cc[:, 0:1])
            nc.vector.scalar_tensor_tensor(
                Bt[:], expT[:, 576:1152], cc[:, 1:2], A[:], op0=OP.mult,
                op1=OP.add,
            )
            nc.vector.scalar_tensor_tensor(
                A[:], expT[:, 1152:1728], cc[:, 2:3], Bt[:], op0=OP.mult,
                op1=OP.add,
            )
            with nc.allow_low_precision(reason="wdyn is consumed in bf16"):
                nc.vector.scalar_tensor_tensor(
                    wdyn[:], expT[:, 1728:2304], cc[:, 3:4], A[:], op0=OP.mult,
                    op1=OP.add,
                )

        def emit_main_group(s, q):
            xv = xvs[s % nxp]
            wdyn = wdyns[s]
            out_t = opool.tile([128, cfg.POUT * 512], F32, tag="outsb",
                               name=f"out_{s}_{q}")
            pss = [mpsum.tile([128, 512], F32, tag="mps", name=f"mps_{s}_{q}_{j}")
                   for j in range(cfg.POUT)]
            # tap-major: per tap the weights are loaded into each column
            # tile once (first matmul), then reused by the other pairs
            for ky in range(3):
                for kx in range(3):
                    wtap = wdyn[:, (ky * 3 + kx) * 64 : (ky * 3 + kx + 1) * 64]
                    st = ky == 0 and kx == 0
                    sp = ky == 2 and kx == 2
                    for j in range(cfg.POUT):
                        pair = q * cfg.POUT + j
                        for hf in range(2):
                            y0 = (2 * pair + hf) * cfg.RPG
                            mm(pss[j][hf * 64 : hf * 64 + 64, :], wtap,
                               xv[:, y0 + ky : y0 + ky + cfg.RPG,
                                  kx : kx + cfg.W],
                               start=st, stop=sp, reuse_w=(j > 0))
            for j in range(cfg.POUT):
                dst = out_t[:, j * 512 : (j + 1) * 512]
                if j % 2 == 0:
                    nc.scalar.activation(dst, pss[j][:], AF.Copy)
                else:
                    nc.vector.tensor_copy(dst, pss[j][:])
                if s == cfg.BL - 1 and q == cfg.NOUT - 1:
                    # drain the very last group per pair so the kernel
                    # tail is one small DMA, not a whole group's worth
                    yvj = y[s].rearrange("c (j r) x -> c j r x",
                                         r=2 * cfg.RPG)
                    for hf in range(2):
                        nc.gpsimd.dma_start(
                            yvj[:, q * cfg.POUT + j,
                                hf * cfg.RPG : (hf + 1) * cfg.RPG, :],
                            out_t[hf * 64 : hf * 64 + 64,
                                  j * 512 : (j + 1) * 512],
                        )
            if not (s == cfg.BL - 1 and q == cfg.NOUT - 1):
                yv = y[s].rearrange("c (j r) x -> c j r x", r=2 * cfg.RPG)
                jj = q * cfg.POUT
                for hf in range(2):
                    dst = yv[:, jj : jj + cfg.POUT,
                             hf * cfg.RPG : (hf + 1) * cfg.RPG, :]
                    nc.gpsimd.dma_start(dst, out_t[hf * 64 : hf * 64 + 64, :])
            if q == cfg.NOUT - 1:
                wdyns.pop(s)

        # ---------- software pipeline ----------
        emit_dma(0)
        # cold weights (heads, experts) land after image 0
        nc.gpsimd.dma_start(blob[:, cfg.NHOT :], wblob[:, cfg.NHOT :])
        emit_pool(0)
        if cfg.BL > 1:
            emit_dma(1)
        emit_conv1(0, tap_major=False)
        emit_conv23(0)
        emit_heads_att_synth(0)
        if cfg.BL > 1:
            emit_pool(1)

        # stage k of sample s+1 (or s+2 for dma/pool) after main group q=k
        def stage_after(s, q):
            if q == min(0, cfg.NOUT - 1):
                if s + 2 < cfg.BL and nxp >= 3:
                    emit_dma(s + 2)
                if s + 1 < cfg.BL:
                    emit_conv1(s + 1, tap_major=True)
            if q == min(1, cfg.NOUT - 1):
                if s + 1 < cfg.BL:
                    emit_conv23(s + 1)
            if q == min(2, cfg.NOUT - 1):
                if s + 1 < cfg.BL:
                    emit_heads_att_synth(s + 1)
            if q == cfg.NOUT - 1:
                if s + 2 < cfg.BL and nxp < 3:
                    emit_dma(s + 2)
                if s + 2 < cfg.BL:
                    emit_pool(s + 2)

        for s in range(cfg.BL):
            for q in range(cfg.NOUT):
                emit_main_group(s, q)
                stage_after(s, q)

    return nc


_NC_CACHE = {}
TRACE = False       # set by test harness to collect an NTFF profile
TRACE_DIR = None    # where to leave the NTFF/perfetto artifacts
LAST_RESULT = None  # BassKernelResults of the most recent kernel() call


def _get_nc(cfg):
    key = (cfg.BL, cfg.Cin, cfg.H, cfg.W)
    if key not in _NC_CACHE:
        nc = build_nc(cfg)
        if not nc.is_finalized():
            nc.finalize()
        _NC_CACHE[key] = nc
    return _NC_CACHE[key]


def pad_images(cfg, x):
    """[n, Cin, H, W] -> zero-padded flat bf16 [n, Cin, NPAD1]."""
    n = x.shape[0]
    xp = np.zeros((n, cfg.Cin, cfg.NPAD1), ml_dtypes.bfloat16)
    xpv = xp[:, :, : cfg.NPAD].reshape(n, cfg.Cin, cfg.Hp, cfg.Wp)
    xpv[:, :, 1 : cfg.H + 1, 1 : cfg.W + 1] = x.astype(ml_dtypes.bfloat16)
    return xp


def kernel(**inputs):
    x = np.asarray(inputs["x"], np.float32)
    B, Cin, H, W = x.shape
    cfg = Cfg(BL=B // 8, Cin=Cin, H=H, W=W)
    nc = _get_nc(cfg)
    wnames = [
        "wm_w1", "wm_b1", "wm_w2", "wm_b2", "tr_w1", "tr_b1", "tr_w2", "tr_b2",
        "tr_w3", "tr_b3", "t1_w", "t1_b", "t2_w", "t2_b", "t3_w", "t3_b",
        "t4_w", "t4_b", "att_cw", "att_cb", "att_fw", "att_fb", "expert_w",
    ]
    ws = {k: np.asarray(inputs[k], np.float32) for k in wnames}
    wm = np.asarray(inputs["wm"], np.float32)
    in_maps = []
    for c in range(8):
        sl = slice(c * cfg.BL, (c + 1) * cfg.BL)
        blob = make_blob(cfg, wm[sl], **ws)
        in_maps.append({"x": pad_images(cfg, x[sl]), "wblob": blob})
    global LAST_RESULT
    kw = {"tmpdir": TRACE_DIR} if (TRACE and TRACE_DIR) else {}
    res = run_bass_kernel_spmd(nc, in_maps, list(range(8)), trace=TRACE, **kw)
    LAST_RESULT = res
    return np.concatenate([res.results[c]["y"] for c in range(8)], axis=0)
